# revision 1
# baseline (speedup 1.0000x reference)
"""Trainium2 Bass kernel for nn_Block_19473381720396 (gnn_message_passing).

Pipeline (per core, SPMD over 8 cores; core c owns output voxels
[c*25000, (c+1)*25000)):
  host: partition edges by out-owner, sort by (in-window, out-row), pack into
        128-slot tiles that never split an out-row run, build int16 index
        streams for dma_gather / dma_scatter_add.
  device, phase 1 (per 7552-slot chunk):
        dma_gather feats rows (bf16, per in-window table)  -> [128,59,128]
        dma_gather w_dw rows by kernel_idx                 -> [128,59,128]
        DVE multiply -> contributions
        DVE is_equal(segloc, iota) -> per-tile one-hot segment selector
        PE matmul selT @ contrib -> per-run segment sums (PSUM)
        ACT copy PSUM -> bf16 comb tile
        dma_scatter_add comb -> acc HBM rows (CCE add; unique targets per
        call, calls serialized by Tile's WAW dep on acc)
  device, phase 2 (per 128-row o-tile):
        acc + b_dw, LayerNorm, transpose, MLP (w1/gelu/w2), + residual feats.
"""
import sys

for _p in ("/opt/trn_rl_repo",):
    if _p not in sys.path:
        sys.path.insert(0, _p)

import numpy as np
import ml_dtypes

import concourse.bacc as bacc
import concourse.bass as bass
import concourse.mybir as mybir
import concourse.tile as tile
from concourse.bass_utils import run_bass_kernel_spmd

# ---------------- problem constants (hardcoded) ----------------
NV = 200000        # voxels
C = 96             # channels
CP = 128           # padded channels (gather elem must be 256B-multiple)
KV = 343           # kernel offsets
NCORE = 8
VPC = NV // NCORE  # 25000 voxels per core
W = 8              # in-windows (int16 gather index limit)

TPC = 59                   # tiles per chunk
CHUNK = TPC * 128          # 7552 slots per chunk (= one gather/scatter call)
CPW = 7                    # chunks per window
SLOTW = CHUNK * CPW        # 52864 slots per window
NCHUNK = W * CPW           # 56 chunks per core
COLS = CHUNK // 16         # 472 int16 idx columns
GARB_BASE = 25088          # scatter garbage rows start (acc)
ACC_ROWS = 32768           # 25088 real+pad rows, garbage up to 32639
NOT = 196                  # output o-tiles of 128 rows (196*128 = 25088)
KPAD = KV + 1              # w table rows (last = zeros)
EPS = 1e-6

TRACE = False
LAST_RESULT = None   # BassKernelResults of last run (for test harness)

_BF16 = ml_dtypes.bfloat16


# ---------------- host-side prep ----------------

def _pack_core(eo, wloc, ek, win):
    """Pack one core's edges (sorted by (win, eo)) into slot arrays.

    Returns dict of per-core arrays:
      gidx  [NCHUNK, CHUNK] int16  in-window row per slot (pad: 0)
      widx  [NCHUNK, CHUNK] int16  w row per slot (pad: KV=zeros row)
      segl  [NCHUNK, 128, TPC] bf16 local segment id per slot
      sidx  [NCHUNK, CHUNK] int16  scatter target per comb slot
    """
    gidx = np.zeros((NCHUNK, CHUNK), np.int16)
    widx = np.full((NCHUNK, CHUNK), KV, np.int16)
    segf = np.full((NCHUNK, CHUNK), 127.0, np.float32)  # segloc per slot
    sidx = np.empty((NCHUNK, CHUNK), np.int16)
    # default scatter target: unique garbage row per slot within a chunk
    garb = GARB_BASE + np.arange(CHUNK, dtype=np.int16)
    sidx[:] = garb[None, :]

    wstart = np.searchsorted(win, np.arange(W + 1))
    for w in range(W):
        s, e = int(wstart[w]), int(wstart[w + 1])
        n = e - s
        if n == 0:
            continue
        eo_w = eo[s:e]
        # run boundaries (eo sorted within window)
        rb = np.flatnonzero(np.diff(eo_w)) + 1
        rs = np.concatenate(([0], rb))            # run starts
        rl = np.diff(np.concatenate((rs, [n])))   # run lengths
        ov = eo_w[rs]                             # run o values
        wl_w = wloc[s:e]
        ek_w = ek[s:e]

        chunk0 = w * CPW
        g_flat = gidx[chunk0:chunk0 + CPW].reshape(-1)
        w_flat = widx[chunk0:chunk0 + CPW].reshape(-1)
        f_flat = segf[chunk0:chunk0 + CPW].reshape(-1)
        s_flat = sidx[chunk0:chunk0 + CPW].reshape(-1)

        pos = 0
        seg = 0          # seg id within current tile
        cur_tile = 0
        rs_l = rs.tolist()
        rl_l = rl.tolist()
        ov_l = ov.tolist()
        for r in range(len(rs_l)):
            L = rl_l[r]
            if L > 128:
                raise RuntimeError("run longer than a tile")
            off = pos & 127
            if off + L > 128:
                pos += 128 - off           # pad to next tile
            t = pos >> 7
            if t != cur_tile:
                cur_tile = t
                seg = 0
            if pos + L > SLOTW:
                raise RuntimeError("window slot capacity exceeded")
            a = rs_l[r]
            g_flat[pos:pos + L] = wl_w[a:a + L]
            w_flat[pos:pos + L] = ek_w[a:a + L]
            f_flat[pos:pos + L] = seg
            # comb slot (partition=seg, tile=t) -> scatter to real o row
            s_flat[t * 128 + seg] = ov_l[r]
            seg += 1
            pos += L
    return gidx, widx, segf, sidx


def _wrap16(a2d):
    """[NCHUNK, CHUNK] -> [NCHUNK, 128, COLS]: slot j -> (j%16, j//16), x8."""
    n = a2d.shape[0]
    w = a2d.reshape(n, COLS, 16).transpose(0, 2, 1)  # [n, 16, COLS]
    return np.tile(w, (1, 8, 1)).astype(np.int16)


def _slotmajor(a2d, dtype):
    """[NCHUNK, CHUNK] -> [NCHUNK, 128, TPC]: slot j -> (j%128, j//128)."""
    n = a2d.shape[0]
    return np.ascontiguousarray(
        a2d.reshape(n, TPC, 128).transpose(0, 2, 1)).astype(dtype)


def _prep(inputs):
    feats = np.asarray(inputs["feats"], np.float32)
    w_dw = np.asarray(inputs["w_dw"], np.float32)
    b_dw = np.asarray(inputs["b_dw"], np.float32)
    ln_w = np.asarray(inputs["ln_w"], np.float32)
    ln_b = np.asarray(inputs["ln_b"], np.float32)
    w1 = np.asarray(inputs["w1"], np.float32)
    b1 = np.asarray(inputs["b1"], np.float32)
    w2 = np.asarray(inputs["w2"], np.float32)
    b2 = np.asarray(inputs["b2"], np.float32)
    in_idx = np.asarray(inputs["in_idx"], np.int64)
    out_idx = np.asarray(inputs["out_idx"], np.int64)
    kernel_idx = np.asarray(inputs["kernel_idx"], np.int64)

    # shared (per-window) feats tables, bf16, channel-padded
    ftabs = []
    for w in range(W):
        t = np.zeros((VPC, CP), _BF16)
        t[:, :C] = feats[w * VPC:(w + 1) * VPC].astype(_BF16)
        ftabs.append(t)
    wtab = np.zeros((KPAD, CP), _BF16)
    wtab[:KV, :C] = w_dw.astype(_BF16)

    iota = np.broadcast_to(np.arange(128, dtype=np.float32), (128, 128))
    iota = np.ascontiguousarray(iota).astype(_BF16)
    ident = np.eye(128, dtype=np.float32).astype(_BF16)
    bdwf = np.ascontiguousarray(np.broadcast_to(b_dw, (128, C)), np.float32)
    gamf = np.ascontiguousarray(np.broadcast_to(ln_w, (128, C)), np.float32)
    betf = np.ascontiguousarray(np.broadcast_to(ln_b, (128, C)), np.float32)
    b2f = np.ascontiguousarray(np.broadcast_to(b2, (128, C)), np.float32)
    b1T = np.ascontiguousarray(b1.reshape(3, 128).T, np.float32)  # [128, 3]
    w1sb = w1.astype(_BF16)                               # [96, 384]
    w2sb = np.ascontiguousarray(
        w2.reshape(3, 128, C).transpose(1, 0, 2)).astype(_BF16)  # [128,3,96]

    owner = out_idx // VPC
    in_maps = []
    for c in range(NCORE):
        sel = np.nonzero(owner == c)[0]
        eo = out_idx[sel] - c * VPC
        ei = in_idx[sel]
        ek = kernel_idx[sel]
        win = ei // VPC
        wloc = ei - win * VPC
        order = np.lexsort((eo, win))
        gidx, widx, segf, sidx = _pack_core(
            eo[order], wloc[order], ek[order], win[order])

        fown = np.zeros((NOT * 128, C), np.float32)
        fown[:VPC] = feats[c * VPC:(c + 1) * VPC]

        m = {
            "wtab": wtab, "iota": iota, "ident": ident,
            "bdwf": bdwf, "gamf": gamf, "betf": betf, "b2f": b2f, "b1T": b1T,
            "w1sb": w1sb, "w2sb": w2sb, "fown": fown,
            "gidx": _wrap16(gidx), "widx": _wrap16(widx),
            "sidx": _wrap16(sidx),
            "segl": _slotmajor(segf, _BF16),
        }
        for w in range(W):
            m[f"ftab{w}"] = ftabs[w]
        in_maps.append(m)
    return in_maps


# ---------------- device program ----------------

def _build():
    import os
    stage = int(os.environ.get("KERNEL_STAGE", "5"))
    nc = bacc.Bacc("TRN2", target_bir_lowering=False, debug=False,
                   dynamic_dma_scratch_size=65536)
    dt = mybir.dt
    ftab = [nc.dram_tensor(f"ftab{w}", [VPC, CP], dt.bfloat16,
                           kind="ExternalInput") for w in range(W)]
    wtab = nc.dram_tensor("wtab", [KPAD, CP], dt.bfloat16, kind="ExternalInput")
    gidx = nc.dram_tensor("gidx", [NCHUNK, 128, COLS], dt.int16, kind="ExternalInput")
    widx = nc.dram_tensor("widx", [NCHUNK, 128, COLS], dt.int16, kind="ExternalInput")
    sidx = nc.dram_tensor("sidx", [NCHUNK, 128, COLS], dt.int16, kind="ExternalInput")
    segl = nc.dram_tensor("segl", [NCHUNK, 128, TPC], dt.bfloat16, kind="ExternalInput")
    iota = nc.dram_tensor("iota", [128, 128], dt.bfloat16, kind="ExternalInput")
    ident = nc.dram_tensor("ident", [128, 128], dt.bfloat16, kind="ExternalInput")
    bdwf = nc.dram_tensor("bdwf", [128, C], dt.float32, kind="ExternalInput")
    gamf = nc.dram_tensor("gamf", [128, C], dt.float32, kind="ExternalInput")
    betf = nc.dram_tensor("betf", [128, C], dt.float32, kind="ExternalInput")
    b2f = nc.dram_tensor("b2f", [128, C], dt.float32, kind="ExternalInput")
    b1T = nc.dram_tensor("b1T", [128, 3], dt.float32, kind="ExternalInput")
    w1sb = nc.dram_tensor("w1sb", [C, 4 * C], dt.bfloat16, kind="ExternalInput")
    w2sb = nc.dram_tensor("w2sb", [128, 3, C], dt.bfloat16, kind="ExternalInput")
    fown = nc.dram_tensor("fown", [NOT * 128, C], dt.float32, kind="ExternalInput")
    acc = nc.dram_tensor("acc", [ACC_ROWS, CP], dt.bfloat16)
    outp = nc.dram_tensor("outp", [NOT * 128, C], dt.float32, kind="ExternalOutput")

    AL = mybir.AluOpType
    AF = mybir.ActivationFunctionType

    with tile.TileContext(nc) as tc:
        with tc.tile_pool(name="const", bufs=1) as cpool, \
             tc.tile_pool(name="sb", bufs=2) as sb, \
             tc.tile_pool(name="sb3", bufs=3) as sb3, \
             tc.tile_pool(name="ps", bufs=2, space="PSUM") as pp:

            # ---- constants into SBUF ----
            iota_t = cpool.tile([128, 128], dt.bfloat16)
            nc.sync.dma_start(out=iota_t[:], in_=iota[:])
            ident_t = cpool.tile([128, 128], dt.bfloat16)
            nc.sync.dma_start(out=ident_t[:], in_=ident[:])
            bdw_t = cpool.tile([128, C], dt.float32)
            nc.sync.dma_start(out=bdw_t[:], in_=bdwf[:])
            gam_t = cpool.tile([128, C], dt.float32)
            nc.sync.dma_start(out=gam_t[:], in_=gamf[:])
            bet_t = cpool.tile([128, C], dt.float32)
            nc.sync.dma_start(out=bet_t[:], in_=betf[:])
            b2_t = cpool.tile([128, C], dt.float32)
            nc.sync.dma_start(out=b2_t[:], in_=b2f[:])
            b1_t = cpool.tile([128, 3], dt.float32)
            nc.sync.dma_start(out=b1_t[:], in_=b1T[:])
            w1_t = cpool.tile([128, 4 * C], dt.bfloat16)
            nc.sync.dma_start(out=w1_t[0:C, :], in_=w1sb[:])
            w2_t = cpool.tile([128, 3, C], dt.bfloat16)
            nc.sync.dma_start(out=w2_t[:], in_=w2sb[:])

            # ---- zero the accumulator ----
            zt = cpool.tile([128, 32, 128], dt.bfloat16)
            nc.vector.memset(zt[:].rearrange("p a c -> p (a c)"), 0)
            accv = acc[:].rearrange("(a p) c -> p a c", p=128)  # [128, 256, 128]
            for z in range(8):
                nc.sync.dma_start(out=accv[:, z * 32:(z + 1) * 32, :], in_=zt[:])

            # ---- phase 1: gather/mult/combine/scatter ----
            for ch in range(NCHUNK):
                w = ch // CPW
                gi_t = sb3.tile([128, COLS], dt.int16, tag="gi")
                nc.sync.dma_start(out=gi_t[:], in_=gidx[ch])
                wi_t = sb3.tile([128, COLS], dt.int16, tag="wi")
                nc.sync.dma_start(out=wi_t[:], in_=widx[ch])
                si_t = sb3.tile([128, COLS], dt.int16, tag="si")
                nc.sync.dma_start(out=si_t[:], in_=sidx[ch])
                sg_t = sb3.tile([128, TPC], dt.bfloat16, tag="sg")
                nc.sync.dma_start(out=sg_t[:], in_=segl[ch])

                g_t = sb.tile([128, TPC, CP], dt.bfloat16, tag="g")
                nc.gpsimd.dma_gather(g_t[:], ftab[w][:], gi_t[:],
                                     CHUNK, CHUNK, CP, single_packet=False)
                w_t = sb.tile([128, TPC, CP], dt.bfloat16, tag="wv")
                nc.gpsimd.dma_gather(w_t[:], wtab[:], wi_t[:],
                                     CHUNK, CHUNK, CP, single_packet=False)
                nc.vector.tensor_tensor(
                    out=g_t[:].rearrange("p a c -> p (a c)"),
                    in0=g_t[:].rearrange("p a c -> p (a c)"),
                    in1=w_t[:].rearrange("p a c -> p (a c)"),
                    op=AL.mult)

                comb_t = sb.tile([128, TPC, C], dt.bfloat16, tag="comb")
                SELB = 16
                for t0 in range(0, TPC, SELB):
                    nb = min(SELB, TPC - t0)
                    sel_t = sb.tile([128, SELB, 128], dt.bfloat16, tag="sel")
                    if stage >= 2:
                        nc.vector.tensor_tensor(
                            out=sel_t[:, 0:nb, :],
                            in0=sg_t[:, t0:t0 + nb].rearrange(
                                "p (t o) -> p t o", o=1).to_broadcast([128, nb, 128]),
                            in1=iota_t[:].rearrange(
                                "p (o f) -> p o f", o=1).to_broadcast([128, nb, 128]),
                            op=AL.is_equal)
                    if stage >= 3:
                        for q0 in range(0, nb, 5):
                            qn = min(5, nb - q0)
                            ps1 = pp.tile([128, 5 * C], dt.float32, tag="ps1")
                            for q in range(qn):
                                t = t0 + q0 + q
                                nc.tensor.matmul(
                                    out=ps1[:, q * C:(q + 1) * C],
                                    lhsT=sel_t[:, q0 + q, :],
                                    rhs=g_t[:, t, 0:C],
                                    start=True, stop=True)
                            nc.scalar.activation(
                                out=comb_t[:, t0 + q0:t0 + q0 + qn, :].rearrange(
                                    "p a c -> p (a c)"),
                                in_=ps1[:, 0:qn * C], func=AF.Copy)
                if stage >= 4:
                    nc.gpsimd.dma_scatter_add(
                        acc[:, 0:C], comb_t[:], si_t[:], CHUNK, CHUNK, C,
                        elem_step=CP, single_packet=False)
                else:
                    # anchor so DCE keeps the stage's work
                    r0 = (ch % NOT) * 128
                    if stage >= 3:
                        nc.sync.dma_start(out=acc[r0:r0 + 128, 0:C],
                                          in_=comb_t[:, 0, :])
                    elif stage == 2:
                        nc.sync.dma_start(out=acc[r0:r0 + 128, 0:96],
                                          in_=sel_t[:, 0, 0:96])
                    else:
                        nc.sync.dma_start(out=acc[r0:r0 + 128, 0:C],
                                          in_=g_t[:, 0, 0:C])

            # ---- phase 2: LN + MLP + residual per o-tile ----
            p2 = int(os.environ.get("KERNEL_P2", "3"))
            for ot in range(NOT if stage >= 5 else 0):
                x_bf = sb3.tile([128, C], dt.bfloat16, tag="xbf")
                nc.sync.dma_start(out=x_bf[:],
                                  in_=acc[ot * 128:(ot + 1) * 128, 0:C])
                f_t = sb3.tile([128, C], dt.float32, tag="fres")
                nc.sync.dma_start(out=f_t[:],
                                  in_=fown[ot * 128:(ot + 1) * 128, :])

                p2ln = int(os.environ.get("KERNEL_P2LN", "9"))
                x32 = sb3.tile([128, C], dt.float32, tag="x32")
                sumx = sb3.tile([128, 1], dt.float32, tag="sumx")
                mu = sb3.tile([128, 1], dt.float32, tag="mu")
                rstd = sb3.tile([128, 1], dt.float32, tag="rstd")
                xln = sb3.tile([128, C], dt.bfloat16, tag="xln")
                if p2ln < 2:
                    nc.vector.tensor_copy(out=x32[:], in_=x_bf[:])
                    nc.vector.tensor_copy(out=xln[:], in_=x32[:])
                else:
                    nc.vector.tensor_copy(out=x32[:], in_=x_bf[:])
                    nc.vector.tensor_tensor(out=x32[:], in0=x32[:],
                                            in1=bdw_t[:], op=AL.add)
                    nc.vector.tensor_reduce(out=sumx[:], in_=x32[:],
                                            axis=mybir.AxisListType.X,
                                            op=AL.add)
                    nc.vector.tensor_scalar_mul(mu[:], sumx[:], 1.0 / C)
                    if p2ln < 3:
                        nc.vector.tensor_copy(out=xln[:], in_=mu[:].to_broadcast([128, C]))
                    else:
                        sq_t = sb3.tile([128, C], dt.float32, tag="sq")
                        ssq = sb3.tile([128, 1], dt.float32, tag="ssq")
                        nc.scalar.activation(out=sq_t[:], in_=x32[:],
                                             func=AF.Square, accum_out=ssq[:])
                        var = sb3.tile([128, 1], dt.float32, tag="var")
                        nc.vector.tensor_scalar_mul(var[:], ssq[:], 1.0 / C)
                        mu2 = sb3.tile([128, 1], dt.float32, tag="mu2")
                        nc.vector.tensor_tensor(out=mu2[:], in0=mu[:],
                                                in1=mu[:], op=AL.mult)
                        nc.vector.tensor_tensor(out=var[:], in0=var[:],
                                                in1=mu2[:], op=AL.subtract)
                        nc.vector.tensor_scalar_add(var[:], var[:], EPS)
                        if p2ln < 4:
                            nc.vector.tensor_copy(
                                out=xln[:], in_=var[:].to_broadcast([128, C]))
                        else:
                            std = sb3.tile([128, 1], dt.float32, tag="std")
                            nc.scalar.activation(out=std[:], in_=var[:],
                                                 func=AF.Sqrt)
                            nc.vector.reciprocal(rstd[:], std[:])
                            xc = sb3.tile([128, C], dt.float32, tag="xc")
                            nc.vector.tensor_scalar(
                                out=xc[:], in0=x32[:], scalar1=mu[:],
                                scalar2=rstd[:],
                                op0=AL.subtract, op1=AL.mult)
                            nc.vector.tensor_tensor(out=xc[:], in0=xc[:],
                                                    in1=gam_t[:], op=AL.mult)
                            nc.vector.tensor_tensor(out=xln[:], in0=xc[:],
                                                    in1=bet_t[:], op=AL.add)
                if p2 == 1:
                    anch = sb3.tile([128, C], dt.float32, tag="anch")
                    nc.vector.tensor_copy(out=anch[:], in_=xln[:])
                    nc.sync.dma_start(
                        out=outp[ot * 128:(ot + 1) * 128, :], in_=anch[:])
                    continue

                pst = pp.tile([128, 128], dt.bfloat16, tag="pst")
                nc.tensor.transpose(out=pst[0:C, :], in_=xln[:, 0:C],
                                    identity=ident_t[:])
                xT = sb3.tile([128, 128], dt.bfloat16, tag="xT")
                nc.scalar.activation(out=xT[0:C, :], in_=pst[0:C, :],
                                     func=AF.Copy)

                psh = pp.tile([128, 3, 128], dt.float32, tag="psh")
                for k in range(3):
                    nc.tensor.matmul(
                        out=psh[:, k, :],
                        lhsT=w1_t[0:C, k * 128:(k + 1) * 128],
                        rhs=xT[0:C, :], start=True, stop=True)
                # h = gelu(w1x + b1): bias add per chunk (b1 transposed layout:
                # psh[p,k,r] corresponds to f1 = k*128+p, so bias differs per
                # (p,k) but is constant along r -> use tensor_scalar per k.
                hT = sb3.tile([128, 3, 128], dt.bfloat16, tag="hT")
                for k in range(3):
                    nc.scalar.activation(
                        out=hT[:, k, :], in_=psh[:, k, :], func=AF.Gelu,
                        bias=b1_t[:, k:k + 1], scale=1.0)

                if p2 == 2:
                    anch = sb3.tile([128, C], dt.float32, tag="anch")
                    nc.vector.tensor_copy(out=anch[:], in_=hT[:, 0, 0:C])
                    nc.sync.dma_start(
                        out=outp[ot * 128:(ot + 1) * 128, :], in_=anch[:])
                    continue

                psx = pp.tile([128, 128], dt.float32, tag="psx")
                for k in range(3):
                    nc.tensor.matmul(out=psx[:, 0:C], lhsT=hT[:, k, :],
                                     rhs=w2_t[:, k, :],
                                     start=(k == 0), stop=(k == 2))
                o32 = sb3.tile([128, C], dt.float32, tag="o32")
                nc.vector.tensor_tensor(out=o32[:], in0=psx[:, 0:C],
                                        in1=b2_t[:], op=AL.add)
                nc.vector.tensor_tensor(out=o32[:], in0=o32[:], in1=f_t[:],
                                        op=AL.add)
                nc.sync.dma_start(out=outp[ot * 128:(ot + 1) * 128, :],
                                  in_=o32[:])
    nc.compile()
    return nc


# ---------------- public entry ----------------

_NC_CACHE = []


def kernel(**inputs):
    global LAST_RESULT
    import os
    ncores_run = int(os.environ.get("KERNEL_NCORES", str(NCORE)))
    in_maps = _prep(inputs)
    if not _NC_CACHE:
        _NC_CACHE.append(_build())
    nc = _NC_CACHE[0]
    kw = {}
    if TRACE:
        kw.update(trace=True)
    res = run_bass_kernel_spmd(nc, in_maps[:ncores_run],
                               core_ids=list(range(ncores_run)), **kw)
    LAST_RESULT = res
    out = np.zeros((NV, C), np.float32)
    for c in range(ncores_run):
        out[c * VPC:(c + 1) * VPC] = res.results[c]["outp"][:VPC]
    return out



# revision 2
# speedup vs baseline: 1.5658x; 1.5658x over previous
"""Trainium2 Bass kernel v2 for nn_Block_19473381720396 (gnn_message_passing).

Key change vs baseline: the baseline spent 9.6ms/core of serialized GPSIMD
descriptor generation (3 SWDGE streams per edge: feats gather + w gather +
scatter-add, ~7.8ns/descriptor).  This version uses ONE SWDGE stream per edge
(the feats gather) and does everything else on PE/DVE/HWDGE:

  host: per core, edges sorted by (in-window, o-block, k); per (w, ot) group
        padded to a dataset-unified tile count; streams: gather idx (int16,
        wrap16 per call), o_local (bf16, slot-major), k (fp16 row).
  device phase 1, per gather call (<=TPC tiles, one in-window):
        HWDGE stream loads; partition_broadcast k -> kb [128, n]
        SWDGE dma_gather feats rows (one 256B desc per slot) -> g [slot,t,c]
        DVE tensor_scalar is_equal(kb, iota_c) -> k one-hot [k', slot]
        PE per tile: 1-3 matmuls oh_c^T @ wtab_c -> w_rows [slot, c] (PSUM)
        DVE mult g*w_rows -> contrib bf16
        DVE is_equal(oloc, iota) -> sel [slot, o_local]
        PE per tile: sel^T @ contrib accumulated into group PSUM [o_local, c]
        DVE per group: acc[:, ot, :] += gps   (SBUF fp32 acc, no HBM acc)
  device phase 2, per o-block: acc + b_dw, LayerNorm, transpose, MLP
        (w1/gelu/w2), + residual feats -> out.

No dma_scatter_add, no acc zero/reload DMA, no per-edge w gather.
"""
import sys

for _p in ("/opt/trn_rl_repo",):
    if _p not in sys.path:
        sys.path.insert(0, _p)

import numpy as np
import ml_dtypes

import concourse.bacc as bacc
import concourse.bass as bass
import concourse.mybir as mybir
import concourse.tile as tile
from concourse.bass_utils import run_bass_kernel_spmd

# ---------------- problem constants ----------------
NV = 200000        # voxels
C = 96             # channels
CP = 128           # padded channels (gather elem must be 256B multiple)
KV = 343           # kernel offsets
NCORE = 8
VPC = NV // NCORE  # 25000 out rows per core
W = 7              # in-windows (int16 gather index limit 32767)
WSZ = 28572        # window size (7*28572 >= 200000)
NOT = 196          # o-blocks of 128 rows (196*128 = 25088)
TPC = 40           # tiles per gather call (5120 slots)
SELB = 16          # sel build batch (tiles)
WPB = 4            # w-expand psum batch (tiles)
SPAN = 4           # oh span (tiles) -> [128, 3, 512]
EPS = 1e-6

TRACE = False
LAST_RESULT = None

_BF16 = ml_dtypes.bfloat16
_FP16 = np.float16


# ---------------- host-side prep ----------------

def _wrap16(a):
    """[n] -> [128, n//16]: slot j -> (j%16, j//16), replicated x8."""
    w = a.reshape(-1, 16).T
    return np.tile(w, (8, 1)).astype(np.int16)


def _prep(inputs):
    feats = np.asarray(inputs["feats"], np.float32)
    w_dw = np.asarray(inputs["w_dw"], np.float32)
    b_dw = np.asarray(inputs["b_dw"], np.float32)
    ln_w = np.asarray(inputs["ln_w"], np.float32)
    ln_b = np.asarray(inputs["ln_b"], np.float32)
    w1 = np.asarray(inputs["w1"], np.float32)
    b1 = np.asarray(inputs["b1"], np.float32)
    w2 = np.asarray(inputs["w2"], np.float32)
    b2 = np.asarray(inputs["b2"], np.float32)
    in_idx = np.asarray(inputs["in_idx"], np.int64)
    out_idx = np.asarray(inputs["out_idx"], np.int64)
    kernel_idx = np.asarray(inputs["kernel_idx"], np.int64)

    # shared tables
    ftabs = []
    for w in range(W):
        lo, hi = w * WSZ, min((w + 1) * WSZ, NV)
        t = np.zeros((WSZ, CP), _BF16)
        t[:hi - lo, :C] = feats[lo:hi].astype(_BF16)
        ftabs.append(t)
    wtab3 = np.zeros((128, 3, C), _BF16)
    wtab3f = np.zeros((384, C), np.float32)
    wtab3f[:KV] = w_dw
    for c in range(3):
        wtab3[:, c, :] = wtab3f[c * 128:(c + 1) * 128].astype(_BF16)

    iota128 = np.broadcast_to(np.arange(128, dtype=np.float32),
                              (128, 128)).astype(_BF16)
    iota128 = np.ascontiguousarray(iota128)
    iotac = np.zeros((128, 3), np.float32)
    for c in range(3):
        iotac[:, c] = np.arange(128) + 128 * c
    ident = np.eye(128, dtype=np.float32).astype(_BF16)
    bdwf = np.ascontiguousarray(np.broadcast_to(b_dw, (128, C)), np.float32)
    gamf = np.ascontiguousarray(np.broadcast_to(ln_w, (128, C)), np.float32)
    betf = np.ascontiguousarray(np.broadcast_to(ln_b, (128, C)), np.float32)
    b2f = np.ascontiguousarray(np.broadcast_to(b2, (128, C)), np.float32)
    b1T = np.ascontiguousarray(b1.reshape(3, 128).T, np.float32)
    w1sb = w1.astype(_BF16)
    w2sb = np.ascontiguousarray(
        w2.reshape(3, 128, C).transpose(1, 0, 2)).astype(_BF16)

    # ---- per-core edge partition + sort ----
    owner = out_idx // VPC
    cores = []
    for c in range(NCORE):
        sel = np.nonzero(owner == c)[0]
        eo = (out_idx[sel] - c * VPC).astype(np.int64)
        ei = in_idx[sel]
        ek = kernel_idx[sel]
        w = ei // WSZ
        wloc = (ei - w * WSZ).astype(np.int64)
        ot = eo >> 7
        ol = eo & 127
        order = np.lexsort((ek, ot, w))
        gid = (w[order] * NOT + ot[order]).astype(np.int64)
        cores.append({
            "gid": gid, "wloc": wloc[order], "ol": ol[order],
            "k": ek[order],
        })

    NG = W * NOT
    counts = np.zeros((NCORE, NG), np.int64)
    for c in range(NCORE):
        counts[c] = np.bincount(cores[c]["gid"], minlength=NG)
    tiles_c = (counts + 127) // 128
    tiles_uni = tiles_c.max(axis=0)          # unified tiles per group
    gtile0 = np.concatenate(([0], np.cumsum(tiles_uni)))  # group tile base
    T_total = int(gtile0[-1])
    S_total = T_total * 128
    # per-window tile counts + call split
    tw = tiles_uni.reshape(W, NOT).sum(1)
    calls = []  # (window, tile0, ntiles)
    tcur = 0
    for w in range(W):
        left = int(tw[w])
        while left > 0:
            n = min(TPC, left)
            calls.append((w, tcur, n))
            tcur += n
            left -= n
    assert tcur == T_total

    # ---- per-core slot arrays ----
    gbase_slots = gtile0[:-1] * 128
    kc_min = np.full((NCORE, T_total), 3, np.int64)
    kc_max = np.full((NCORE, T_total), -1, np.int64)
    per_core = []
    for c in range(NCORE):
        d = cores[c]
        gid = d["gid"]
        cnt = counts[c]
        starts = np.concatenate(([0], np.cumsum(cnt)))[:-1]
        rank = np.arange(len(gid)) - starts[gid]
        slot = gbase_slots[gid] + rank
        gidx = np.zeros(S_total, np.int16)
        gidx[slot] = d["wloc"]
        kvs = np.full(S_total, -1, np.int64)   # -1 = pad, fill later
        kvs[slot] = d["k"]
        olv = np.full(S_total, 255, np.int64)
        olv[slot] = d["ol"]
        # per-tile real k-chunk min/max
        kc = np.where(kvs >= 0, kvs >> 7, -1).reshape(T_total, 128)
        has = kc.max(1)
        kmax = kc.max(1)
        kcpos = np.where(kc < 0, 3, kc)
        kmin = kcpos.min(1)
        valid = has >= 0
        kc_min[c, valid] = kmin[valid]
        kc_max[c, valid] = kmax[valid]
        per_core.append({"gidx": gidx, "kvs": kvs, "olv": olv})

    cmin = kc_min.min(0)
    cmax = kc_max.max(0)
    empty = cmax < 0
    cmin[empty] = 2
    cmax[empty] = 2
    # pad k per tile = 128*cmin (valid wtab row; killed by sel anyway)
    padk = (cmin * 128).astype(np.int64)
    padk[empty] = KV  # zeros row
    tile_chunks = [tuple(range(int(cmin[t]), int(cmax[t]) + 1))
                   for t in range(T_total)]

    # group schedule: per tile -> (group-start?, group-end?, ot)
    gs = np.zeros(T_total, np.bool_)
    ge = np.zeros(T_total, np.bool_)
    got = np.zeros(T_total, np.int64)
    for g in range(NG):
        t0, t1 = int(gtile0[g]), int(gtile0[g + 1])
        if t1 > t0:
            gs[t0] = True
            ge[t1 - 1] = True
            got[t0:t1] = g % NOT
    sched = {
        "calls": calls, "tile_chunks": tile_chunks,
        "gs": gs, "ge": ge, "got": got, "T_total": T_total,
    }

    in_maps = []
    for c in range(NCORE):
        d = per_core[c]
        kvs = d["kvs"].reshape(T_total, 128)
        padm = kvs < 0
        kvs = np.where(padm, padk[:, None], kvs)
        kval = kvs.reshape(-1).astype(_FP16)[None, :]        # [1, S]
        oloc = np.ascontiguousarray(
            d["olv"].astype(np.float32).reshape(T_total, 128).T).astype(_BF16)
        # wrap idx per call
        gw = []
        for (w, t0, nt) in calls:
            seg = d["gidx"][t0 * 128:(t0 + nt) * 128]
            gw.append(_wrap16(seg))
        gidxw = np.ascontiguousarray(np.concatenate(gw, axis=1))  # [128, S/16]

        fown = np.zeros((NOT * 128, C), np.float32)
        fown[:VPC] = feats[c * VPC:(c + 1) * VPC]

        m = {
            "wtab3": wtab3, "iota128": iota128, "iotac": iotac, "ident": ident,
            "bdwf": bdwf, "gamf": gamf, "betf": betf, "b2f": b2f, "b1T": b1T,
            "w1sb": w1sb, "w2sb": w2sb, "fown": fown,
            "gidxw": gidxw, "kval": kval, "oloc": oloc,
        }
        for w in range(W):
            m[f"ftab{w}"] = ftabs[w]
        in_maps.append(m)
    return in_maps, sched


# ---------------- device program ----------------

def _build(sched):
    nc = bacc.Bacc("TRN2", target_bir_lowering=False, debug=False,
                   dynamic_dma_scratch_size=65536)
    dt = mybir.dt
    AL = mybir.AluOpType
    AF = mybir.ActivationFunctionType
    T_total = sched["T_total"]
    S_total = T_total * 128
    calls = sched["calls"]
    tile_chunks = sched["tile_chunks"]
    gs, ge, got = sched["gs"], sched["ge"], sched["got"]

    ftab = [nc.dram_tensor(f"ftab{w}", [WSZ, CP], dt.bfloat16,
                           kind="ExternalInput") for w in range(W)]
    wtab3 = nc.dram_tensor("wtab3", [128, 3, C], dt.bfloat16, kind="ExternalInput")
    iota128 = nc.dram_tensor("iota128", [128, 128], dt.bfloat16, kind="ExternalInput")
    iotac = nc.dram_tensor("iotac", [128, 3], dt.float32, kind="ExternalInput")
    ident = nc.dram_tensor("ident", [128, 128], dt.bfloat16, kind="ExternalInput")
    bdwf = nc.dram_tensor("bdwf", [128, C], dt.float32, kind="ExternalInput")
    gamf = nc.dram_tensor("gamf", [128, C], dt.float32, kind="ExternalInput")
    betf = nc.dram_tensor("betf", [128, C], dt.float32, kind="ExternalInput")
    b2f = nc.dram_tensor("b2f", [128, C], dt.float32, kind="ExternalInput")
    b1T = nc.dram_tensor("b1T", [128, 3], dt.float32, kind="ExternalInput")
    w1sb = nc.dram_tensor("w1sb", [C, 4 * C], dt.bfloat16, kind="ExternalInput")
    w2sb = nc.dram_tensor("w2sb", [128, 3, C], dt.bfloat16, kind="ExternalInput")
    fown = nc.dram_tensor("fown", [NOT * 128, C], dt.float32, kind="ExternalInput")
    gidxw = nc.dram_tensor("gidxw", [128, S_total // 16], dt.int16,
                           kind="ExternalInput")
    kval = nc.dram_tensor("kval", [1, S_total], dt.float16, kind="ExternalInput")
    olocd = nc.dram_tensor("oloc", [128, T_total], dt.bfloat16,
                           kind="ExternalInput")
    outp = nc.dram_tensor("outp", [NOT * 128, C], dt.float32,
                          kind="ExternalOutput")

    with tile.TileContext(nc) as tc:
        with tc.tile_pool(name="const", bufs=1) as cpool, \
             tc.tile_pool(name="st", bufs=3) as st, \
             tc.tile_pool(name="gkb", bufs=2) as gkb, \
             tc.tile_pool(name="ohp", bufs=2) as ohp, \
             tc.tile_pool(name="cbp", bufs=2) as cbp, \
             tc.tile_pool(name="selp", bufs=2) as selp, \
             tc.tile_pool(name="p2", bufs=2) as p2, \
             tc.tile_pool(name="wpsp", bufs=2, space="PSUM") as wpsp, \
             tc.tile_pool(name="gpsp", bufs=3, space="PSUM") as gpsp, \
             tc.tile_pool(name="p2ps", bufs=1, space="PSUM") as p2ps:

            # ---- constants ----
            wt = cpool.tile([128, 3, C], dt.bfloat16)
            nc.sync.dma_start(out=wt[:], in_=wtab3[:])
            io128 = cpool.tile([128, 128], dt.bfloat16)
            nc.sync.dma_start(out=io128[:], in_=iota128[:])
            ioc = cpool.tile([128, 3], dt.float32)
            nc.sync.dma_start(out=ioc[:], in_=iotac[:])
            ident_t = cpool.tile([128, 128], dt.bfloat16)
            nc.sync.dma_start(out=ident_t[:], in_=ident[:])
            bdw_t = cpool.tile([128, C], dt.float32)
            nc.sync.dma_start(out=bdw_t[:], in_=bdwf[:])
            gam_t = cpool.tile([128, C], dt.float32)
            nc.sync.dma_start(out=gam_t[:], in_=gamf[:])
            bet_t = cpool.tile([128, C], dt.float32)
            nc.sync.dma_start(out=bet_t[:], in_=betf[:])
            b2_t = cpool.tile([128, C], dt.float32)
            nc.sync.dma_start(out=b2_t[:], in_=b2f[:])
            b1_t = cpool.tile([128, 3], dt.float32)
            nc.sync.dma_start(out=b1_t[:], in_=b1T[:])
            w1_t = cpool.tile([128, 4 * C], dt.bfloat16)
            nc.sync.dma_start(out=w1_t[0:C, :], in_=w1sb[:])
            w2_t = cpool.tile([128, 3, C], dt.bfloat16)
            nc.sync.dma_start(out=w2_t[:], in_=w2sb[:])

            # ---- SBUF fp32 accumulator ----
            acc_t = cpool.tile([128, NOT, C], dt.float32)
            nc.vector.memset(acc_t[:].rearrange("p a c -> p (a c)"), 0)

            # ---- phase 1 ----
            cur_gps = [None]

            for (wnd, t0, nt) in calls:
                n = nt * 128
                gi_t = st.tile([128, nt * 8], dt.int16, tag="gi")
                nc.sync.dma_start(out=gi_t[:],
                                  in_=gidxw[:, t0 * 8:(t0 + nt) * 8])
                ol_t = st.tile([128, nt], dt.bfloat16, tag="ol")
                nc.sync.dma_start(out=ol_t[:], in_=olocd[:, t0:t0 + nt])
                g_t = gkb.tile([128, TPC, CP], dt.bfloat16, tag="g")
                nc.gpsimd.dma_gather(g_t[:, 0:nt, :], ftab[wnd][:], gi_t[:],
                                     n, n, CP, single_packet=False)
                cb_t = cbp.tile([128, TPC, C], dt.bfloat16, tag="cb")

                kb_t = gkb.tile([128, TPC * 128], dt.float16, tag="kb")
                nc.sync.dma_start(out=kb_t[0:1, 0:n],
                                  in_=kval[:, t0 * 128:t0 * 128 + n])
                nc.gpsimd.partition_broadcast(kb_t[:, 0:n], kb_t[0:1, 0:n],
                                              channels=128)

                # per span: k one-hots, w-expand, mult
                for s0 in range(0, nt, SPAN):
                    sn = min(SPAN, nt - s0)
                    oh_t = ohp.tile([128, 3, SPAN * 128], dt.bfloat16, tag="oh")
                    need = set()
                    for j in range(sn):
                        need.update(tile_chunks[t0 + s0 + j])
                    for c in sorted(need):
                        nc.vector.tensor_scalar(
                            out=oh_t[:, c, 0:sn * 128],
                            in0=kb_t[:, (s0) * 128:(s0 + sn) * 128],
                            scalar1=ioc[:, c:c + 1], scalar2=None,
                            op0=AL.is_equal)
                    for b0 in range(s0, s0 + sn, WPB):
                        bn = min(WPB, s0 + sn - b0)
                        wps = wpsp.tile([128, WPB, C], dt.float32, tag="wps")
                        for j in range(bn):
                            tj = t0 + b0 + j
                            chunks = tile_chunks[tj]
                            for ci, cch in enumerate(chunks):
                                nc.tensor.matmul(
                                    out=wps[:, j, :],
                                    lhsT=oh_t[:, cch,
                                              (b0 - s0 + j) * 128:
                                              (b0 - s0 + j + 1) * 128],
                                    rhs=wt[:, cch, :],
                                    start=(ci == 0),
                                    stop=(ci == len(chunks) - 1))
                        nc.vector.tensor_tensor(
                            out=cb_t[:, b0:b0 + bn, :],
                            in0=g_t[:, b0:b0 + bn, 0:C],
                            in1=wps[:, 0:bn, :],
                            op=AL.mult)

                # sel + group matmuls
                for s0 in range(0, nt, SELB):
                    sn = min(SELB, nt - s0)
                    sel_t = selp.tile([128, SELB, 128], dt.bfloat16, tag="sel")
                    nc.vector.tensor_tensor(
                        out=sel_t[:, 0:sn, :],
                        in0=ol_t[:, s0:s0 + sn].rearrange(
                            "p (t o) -> p t o", o=1).to_broadcast([128, sn, 128]),
                        in1=io128[:].rearrange(
                            "p (o f) -> p o f", o=1).to_broadcast([128, sn, 128]),
                        op=AL.is_equal)
                    for j in range(sn):
                        tj = t0 + s0 + j
                        if gs[tj]:
                            cur_gps[0] = gpsp.tile([128, C], dt.float32,
                                                   tag="gps", name="gps")
                        gps = cur_gps[0]
                        # find cb tile holding tj: cb pool tiles are per-call
                        nc.tensor.matmul(out=gps[:],
                                         lhsT=sel_t[:, j, :],
                                         rhs=cb_t[:, s0 + j, :],
                                         start=bool(gs[tj]),
                                         stop=bool(ge[tj]))
                        if ge[tj]:
                            ot = int(got[tj])
                            nc.vector.tensor_tensor(
                                out=acc_t[:, ot, :], in0=acc_t[:, ot, :],
                                in1=gps[:], op=AL.add)

            # ---- phase 2 ----
            for ot in range(NOT):
                f_t = p2.tile([128, C], dt.float32, tag="fres")
                nc.sync.dma_start(out=f_t[:],
                                  in_=fown[ot * 128:(ot + 1) * 128, :])
                x32 = p2.tile([128, C], dt.float32, tag="x32")
                nc.vector.tensor_tensor(out=x32[:], in0=acc_t[:, ot, :],
                                        in1=bdw_t[:], op=AL.add)
                sumx = p2.tile([128, 1], dt.float32, tag="sumx")
                nc.vector.tensor_reduce(out=sumx[:], in_=x32[:],
                                        axis=mybir.AxisListType.X, op=AL.add)
                mu = p2.tile([128, 1], dt.float32, tag="mu")
                nc.vector.tensor_scalar_mul(mu[:], sumx[:], 1.0 / C)
                sq_t = p2.tile([128, C], dt.float32, tag="sq")
                ssq = p2.tile([128, 1], dt.float32, tag="ssq")
                nc.scalar.activation(out=sq_t[:], in_=x32[:],
                                     func=AF.Square, accum_out=ssq[:])
                var = p2.tile([128, 1], dt.float32, tag="var")
                nc.vector.tensor_scalar_mul(var[:], ssq[:], 1.0 / C)
                mu2 = p2.tile([128, 1], dt.float32, tag="mu2")
                nc.vector.tensor_tensor(out=mu2[:], in0=mu[:], in1=mu[:],
                                        op=AL.mult)
                nc.vector.tensor_tensor(out=var[:], in0=var[:], in1=mu2[:],
                                        op=AL.subtract)
                nc.vector.tensor_scalar_add(var[:], var[:], EPS)
                std = p2.tile([128, 1], dt.float32, tag="std")
                nc.scalar.activation(out=std[:], in_=var[:], func=AF.Sqrt)
                rstd = p2.tile([128, 1], dt.float32, tag="rstd")
                nc.vector.reciprocal(rstd[:], std[:])
                xc = p2.tile([128, C], dt.float32, tag="xc")
                nc.vector.tensor_scalar(out=xc[:], in0=x32[:], scalar1=mu[:],
                                        scalar2=rstd[:],
                                        op0=AL.subtract, op1=AL.mult)
                nc.vector.tensor_tensor(out=xc[:], in0=xc[:], in1=gam_t[:],
                                        op=AL.mult)
                xln = p2.tile([128, C], dt.bfloat16, tag="xln")
                nc.vector.tensor_tensor(out=xln[:], in0=xc[:], in1=bet_t[:],
                                        op=AL.add)

                pst = p2ps.tile([128, 128], dt.bfloat16, tag="pst")
                nc.tensor.transpose(out=pst[0:C, :], in_=xln[:, 0:C],
                                    identity=ident_t[:])
                xT = p2.tile([128, 128], dt.bfloat16, tag="xT")
                nc.scalar.activation(out=xT[0:C, :], in_=pst[0:C, :],
                                     func=AF.Copy)

                psh = p2ps.tile([128, 3, 128], dt.float32, tag="psh")
                for k in range(3):
                    nc.tensor.matmul(
                        out=psh[:, k, :],
                        lhsT=w1_t[0:C, k * 128:(k + 1) * 128],
                        rhs=xT[0:C, :], start=True, stop=True)
                hT = p2.tile([128, 3, 128], dt.bfloat16, tag="hT")
                for k in range(3):
                    nc.scalar.activation(
                        out=hT[:, k, :], in_=psh[:, k, :], func=AF.Gelu,
                        bias=b1_t[:, k:k + 1], scale=1.0)

                psx = p2ps.tile([128, 128], dt.float32, tag="psx")
                for k in range(3):
                    nc.tensor.matmul(out=psx[:, 0:C], lhsT=hT[:, k, :],
                                     rhs=w2_t[:, k, :],
                                     start=(k == 0), stop=(k == 2))
                o32 = p2.tile([128, C], dt.float32, tag="o32")
                nc.vector.tensor_tensor(out=o32[:], in0=psx[:, 0:C],
                                        in1=b2_t[:], op=AL.add)
                nc.vector.tensor_tensor(out=o32[:], in0=o32[:], in1=f_t[:],
                                        op=AL.add)
                nc.sync.dma_start(out=outp[ot * 128:(ot + 1) * 128, :],
                                  in_=o32[:])
    nc.compile()
    return nc


# ---------------- public entry ----------------

_NC_CACHE = {}


def kernel(**inputs):
    global LAST_RESULT
    import os
    ncores_run = int(os.environ.get("KERNEL_NCORES", str(NCORE)))
    in_maps, sched = _prep(inputs)
    key = (sched["T_total"], tuple(sched["calls"]),
           tuple(sched["tile_chunks"]),
           sched["gs"].tobytes(), sched["ge"].tobytes(),
           sched["got"].tobytes())
    key = hash(key)
    if key not in _NC_CACHE:
        _NC_CACHE[key] = _build(sched)
    nc = _NC_CACHE[key]
    kw = {}
    if TRACE:
        kw.update(trace=True)
    res = run_bass_kernel_spmd(nc, in_maps[:ncores_run],
                               core_ids=list(range(ncores_run)), **kw)
    LAST_RESULT = res
    out = np.zeros((NV, C), np.float32)
    for c in range(ncores_run):
        out[c * VPC:(c + 1) * VPC] = res.results[c]["outp"][:VPC]
    return out


# revision 3
# speedup vs baseline: 1.7531x; 1.1196x over previous
"""Trainium2 Bass kernel v2 for nn_Block_19473381720396 (gnn_message_passing).

Key change vs baseline: the baseline spent 9.6ms/core of serialized GPSIMD
descriptor generation (3 SWDGE streams per edge: feats gather + w gather +
scatter-add, ~7.8ns/descriptor).  This version uses ONE SWDGE stream per edge
(the feats gather) and does everything else on PE/DVE/HWDGE:

  host: per core, edges sorted by (in-window, o-block, k); per (w, ot) group
        padded to a dataset-unified tile count; streams: gather idx (int16,
        wrap16 per call), o_local (bf16, slot-major), k (fp16 row).
  device phase 1, per gather call (<=TPC tiles, one in-window):
        HWDGE stream loads; partition_broadcast k -> kb [128, n]
        SWDGE dma_gather feats rows (one 256B desc per slot) -> g [slot,t,c]
        DVE tensor_scalar is_equal(kb, iota_c) -> k one-hot [k', slot]
        PE per tile: 1-3 matmuls oh_c^T @ wtab_c -> w_rows [slot, c] (PSUM)
        DVE mult g*w_rows -> contrib bf16
        DVE is_equal(oloc, iota) -> sel [slot, o_local]
        PE per tile: sel^T @ contrib accumulated into group PSUM [o_local, c]
        DVE per group: acc[:, ot, :] += gps   (SBUF fp32 acc, no HBM acc)
  device phase 2, per o-block: acc + b_dw, LayerNorm, transpose, MLP
        (w1/gelu/w2), + residual feats -> out.

No dma_scatter_add, no acc zero/reload DMA, no per-edge w gather.
"""
import sys

for _p in ("/opt/trn_rl_repo",):
    if _p not in sys.path:
        sys.path.insert(0, _p)

import numpy as np
import ml_dtypes

import concourse.bacc as bacc
import concourse.bass as bass
import concourse.mybir as mybir
import concourse.tile as tile
from concourse.bass_utils import run_bass_kernel_spmd

# ---------------- problem constants ----------------
NV = 200000        # voxels
C = 96             # channels
CP = 128           # padded channels (gather elem must be 256B multiple)
KV = 343           # kernel offsets
NCORE = 8
VPC = NV // NCORE  # 25000 out rows per core
W = 7              # in-windows (int16 gather index limit 32767)
WSZ = 28572        # window size (7*28572 >= 200000)
NOT = 196          # o-blocks of 128 rows (196*128 = 25088)
TPC = 40           # tiles per gather call (5120 slots)
SELB = 16          # sel build batch (tiles)
WPB = 4            # w-expand psum batch (tiles)
SPAN = 4           # oh span (tiles) -> [128, 3, 512]
EPS = 1e-6

TRACE = False
LAST_RESULT = None

_BF16 = ml_dtypes.bfloat16
_FP16 = np.float16


# ---------------- host-side prep ----------------

def _wrap16(a):
    """[n] -> [128, n//16]: slot j -> (j%16, j//16), replicated x8."""
    w = a.reshape(-1, 16).T
    return np.tile(w, (8, 1)).astype(np.int16)


def _prep(inputs):
    feats = np.asarray(inputs["feats"], np.float32)
    w_dw = np.asarray(inputs["w_dw"], np.float32)
    b_dw = np.asarray(inputs["b_dw"], np.float32)
    ln_w = np.asarray(inputs["ln_w"], np.float32)
    ln_b = np.asarray(inputs["ln_b"], np.float32)
    w1 = np.asarray(inputs["w1"], np.float32)
    b1 = np.asarray(inputs["b1"], np.float32)
    w2 = np.asarray(inputs["w2"], np.float32)
    b2 = np.asarray(inputs["b2"], np.float32)
    in_idx = np.asarray(inputs["in_idx"], np.int64)
    out_idx = np.asarray(inputs["out_idx"], np.int64)
    kernel_idx = np.asarray(inputs["kernel_idx"], np.int64)

    # shared tables
    ftabs = []
    for w in range(W):
        lo, hi = w * WSZ, min((w + 1) * WSZ, NV)
        t = np.zeros((WSZ, CP), _BF16)
        t[:hi - lo, :C] = feats[lo:hi].astype(_BF16)
        ftabs.append(t)
    wtab3 = np.zeros((128, 3, C), _BF16)
    wtab3f = np.zeros((384, C), np.float32)
    wtab3f[:KV] = w_dw
    for c in range(3):
        wtab3[:, c, :] = wtab3f[c * 128:(c + 1) * 128].astype(_BF16)

    iota128 = np.broadcast_to(np.arange(128, dtype=np.float32),
                              (128, 128)).astype(_BF16)
    iota128 = np.ascontiguousarray(iota128)
    iotac = np.zeros((128, 3), np.float32)
    for c in range(3):
        iotac[:, c] = np.arange(128) + 128 * c
    ident = np.eye(128, dtype=np.float32).astype(_BF16)
    bdwf = np.ascontiguousarray(np.broadcast_to(b_dw, (128, C)), np.float32)
    gamf = np.ascontiguousarray(np.broadcast_to(ln_w, (128, C)), np.float32)
    betf = np.ascontiguousarray(np.broadcast_to(ln_b, (128, C)), np.float32)
    b2f = np.ascontiguousarray(np.broadcast_to(b2, (128, C)), np.float32)
    b1T = np.ascontiguousarray(b1.reshape(3, 128).T, np.float32)
    w1sb = w1.astype(_BF16)
    w2sb = np.ascontiguousarray(
        w2.reshape(3, 128, C).transpose(1, 0, 2)).astype(_BF16)

    # ---- per-core edge partition + sort ----
    owner = out_idx // VPC
    cores = []
    for c in range(NCORE):
        sel = np.nonzero(owner == c)[0]
        eo = (out_idx[sel] - c * VPC).astype(np.int64)
        ei = in_idx[sel]
        ek = kernel_idx[sel]
        w = ei // WSZ
        wloc = (ei - w * WSZ).astype(np.int64)
        ot = eo >> 7
        ol = eo & 127
        order = np.lexsort((ek, ot, w))
        gid = (w[order] * NOT + ot[order]).astype(np.int64)
        cores.append({
            "gid": gid, "wloc": wloc[order], "ol": ol[order],
            "k": ek[order],
        })

    NG = W * NOT
    counts = np.zeros((NCORE, NG), np.int64)
    for c in range(NCORE):
        counts[c] = np.bincount(cores[c]["gid"], minlength=NG)
    tiles_c = (counts + 127) // 128
    tiles_uni = tiles_c.max(axis=0)          # unified tiles per group
    gtile0 = np.concatenate(([0], np.cumsum(tiles_uni)))  # group tile base
    T_total = int(gtile0[-1])
    S_total = T_total * 128
    # per-window tile counts + call split
    tw = tiles_uni.reshape(W, NOT).sum(1)
    calls = []  # (window, tile0, ntiles)
    tcur = 0
    for w in range(W):
        left = int(tw[w])
        while left > 0:
            n = min(TPC, left)
            calls.append((w, tcur, n))
            tcur += n
            left -= n
    assert tcur == T_total

    # ---- per-core slot arrays ----
    gbase_slots = gtile0[:-1] * 128
    kc_min = np.full((NCORE, T_total), 3, np.int64)
    kc_max = np.full((NCORE, T_total), -1, np.int64)
    per_core = []
    for c in range(NCORE):
        d = cores[c]
        gid = d["gid"]
        cnt = counts[c]
        starts = np.concatenate(([0], np.cumsum(cnt)))[:-1]
        rank = np.arange(len(gid)) - starts[gid]
        slot = gbase_slots[gid] + rank
        gidx = np.zeros(S_total, np.int16)
        gidx[slot] = d["wloc"]
        kvs = np.full(S_total, -1, np.int64)   # -1 = pad, fill later
        kvs[slot] = d["k"]
        olv = np.full(S_total, 255, np.int64)
        olv[slot] = d["ol"]
        # per-tile real k-chunk min/max
        kc = np.where(kvs >= 0, kvs >> 7, -1).reshape(T_total, 128)
        has = kc.max(1)
        kmax = kc.max(1)
        kcpos = np.where(kc < 0, 3, kc)
        kmin = kcpos.min(1)
        valid = has >= 0
        kc_min[c, valid] = kmin[valid]
        kc_max[c, valid] = kmax[valid]
        per_core.append({"gidx": gidx, "kvs": kvs, "olv": olv})

    cmin = kc_min.min(0)
    cmax = kc_max.max(0)
    empty = cmax < 0
    cmin[empty] = 2
    cmax[empty] = 2
    # pad k per tile = 128*cmin (valid wtab row; killed by sel anyway)
    padk = (cmin * 128).astype(np.int64)
    padk[empty] = KV  # zeros row
    tile_chunks = [tuple(range(int(cmin[t]), int(cmax[t]) + 1))
                   for t in range(T_total)]

    # group schedule: per tile -> (group-start?, group-end?, ot)
    gs = np.zeros(T_total, np.bool_)
    ge = np.zeros(T_total, np.bool_)
    got = np.zeros(T_total, np.int64)
    for g in range(NG):
        t0, t1 = int(gtile0[g]), int(gtile0[g + 1])
        if t1 > t0:
            gs[t0] = True
            ge[t1 - 1] = True
            got[t0:t1] = g % NOT
    sched = {
        "calls": calls, "tile_chunks": tile_chunks,
        "gs": gs, "ge": ge, "got": got, "T_total": T_total,
    }

    in_maps = []
    for c in range(NCORE):
        d = per_core[c]
        kvs = d["kvs"].reshape(T_total, 128)
        padm = kvs < 0
        kvs = np.where(padm, padk[:, None], kvs)
        kval = kvs.reshape(-1).astype(_FP16)[None, :]        # [1, S]
        oloc = np.ascontiguousarray(
            d["olv"].astype(np.float32).reshape(T_total, 128).T).astype(_BF16)
        # wrap idx per call
        gw = []
        for (w, t0, nt) in calls:
            seg = d["gidx"][t0 * 128:(t0 + nt) * 128]
            gw.append(_wrap16(seg))
        gidxw = np.ascontiguousarray(np.concatenate(gw, axis=1))  # [128, S/16]

        fown = np.zeros((NOT * 128, C), np.float32)
        fown[:VPC] = feats[c * VPC:(c + 1) * VPC]

        m = {
            "wtab3": wtab3, "iota128": iota128, "iotac": iotac, "ident": ident,
            "bdwf": bdwf, "gamf": gamf, "betf": betf, "b2f": b2f, "b1T": b1T,
            "w1sb": w1sb, "w2sb": w2sb, "fown": fown,
            "gidxw": gidxw, "kval": kval, "oloc": oloc,
        }
        for w in range(W):
            m[f"ftab{w}"] = ftabs[w]
        in_maps.append(m)
    return in_maps, sched


# ---------------- device program ----------------

def _build(sched):
    nc = bacc.Bacc("TRN2", target_bir_lowering=False, debug=False,
                   dynamic_dma_scratch_size=65536)
    dt = mybir.dt
    AL = mybir.AluOpType
    AF = mybir.ActivationFunctionType
    T_total = sched["T_total"]
    S_total = T_total * 128
    calls = sched["calls"]
    tile_chunks = sched["tile_chunks"]
    gs, ge, got = sched["gs"], sched["ge"], sched["got"]

    ftab = [nc.dram_tensor(f"ftab{w}", [WSZ, CP], dt.bfloat16,
                           kind="ExternalInput") for w in range(W)]
    wtab3 = nc.dram_tensor("wtab3", [128, 3, C], dt.bfloat16, kind="ExternalInput")
    iota128 = nc.dram_tensor("iota128", [128, 128], dt.bfloat16, kind="ExternalInput")
    iotac = nc.dram_tensor("iotac", [128, 3], dt.float32, kind="ExternalInput")
    ident = nc.dram_tensor("ident", [128, 128], dt.bfloat16, kind="ExternalInput")
    bdwf = nc.dram_tensor("bdwf", [128, C], dt.float32, kind="ExternalInput")
    gamf = nc.dram_tensor("gamf", [128, C], dt.float32, kind="ExternalInput")
    betf = nc.dram_tensor("betf", [128, C], dt.float32, kind="ExternalInput")
    b2f = nc.dram_tensor("b2f", [128, C], dt.float32, kind="ExternalInput")
    b1T = nc.dram_tensor("b1T", [128, 3], dt.float32, kind="ExternalInput")
    w1sb = nc.dram_tensor("w1sb", [C, 4 * C], dt.bfloat16, kind="ExternalInput")
    w2sb = nc.dram_tensor("w2sb", [128, 3, C], dt.bfloat16, kind="ExternalInput")
    fown = nc.dram_tensor("fown", [NOT * 128, C], dt.float32, kind="ExternalInput")
    gidxw = nc.dram_tensor("gidxw", [128, S_total // 16], dt.int16,
                           kind="ExternalInput")
    kval = nc.dram_tensor("kval", [1, S_total], dt.float16, kind="ExternalInput")
    olocd = nc.dram_tensor("oloc", [128, T_total], dt.bfloat16,
                           kind="ExternalInput")
    outp = nc.dram_tensor("outp", [NOT * 128, C], dt.float32,
                          kind="ExternalOutput")

    with tile.TileContext(nc) as tc:
        with tc.tile_pool(name="const", bufs=1) as cpool, \
             tc.tile_pool(name="st", bufs=3) as st, \
             tc.tile_pool(name="gkb", bufs=2) as gkb, \
             tc.tile_pool(name="kvp", bufs=2) as kvp, \
             tc.tile_pool(name="ohp", bufs=2) as ohp, \
             tc.tile_pool(name="cbp", bufs=2) as cbp, \
             tc.tile_pool(name="selp", bufs=2) as selp, \
             tc.tile_pool(name="p2", bufs=2) as p2, \
             tc.tile_pool(name="wpsp", bufs=2, space="PSUM") as wpsp, \
             tc.tile_pool(name="gpsp", bufs=2, space="PSUM") as gpsp, \
             tc.tile_pool(name="kbpp", bufs=1, space="PSUM") as kbpp, \
             tc.tile_pool(name="p2ps", bufs=1, space="PSUM") as p2ps:

            # ---- constants ----
            wt = cpool.tile([128, 3, C], dt.bfloat16)
            nc.sync.dma_start(out=wt[:], in_=wtab3[:])
            io128 = cpool.tile([128, 128], dt.bfloat16)
            nc.sync.dma_start(out=io128[:], in_=iota128[:])
            ioc = cpool.tile([128, 3], dt.float32)
            nc.sync.dma_start(out=ioc[:], in_=iotac[:])
            ones1 = cpool.tile([1, 128], dt.float16)
            nc.vector.memset(ones1[:], 1.0)
            ident_t = cpool.tile([128, 128], dt.bfloat16)
            nc.sync.dma_start(out=ident_t[:], in_=ident[:])
            bdw_t = cpool.tile([128, C], dt.float32)
            nc.sync.dma_start(out=bdw_t[:], in_=bdwf[:])
            gam_t = cpool.tile([128, C], dt.float32)
            nc.sync.dma_start(out=gam_t[:], in_=gamf[:])
            bet_t = cpool.tile([128, C], dt.float32)
            nc.sync.dma_start(out=bet_t[:], in_=betf[:])
            b2_t = cpool.tile([128, C], dt.float32)
            nc.sync.dma_start(out=b2_t[:], in_=b2f[:])
            b1_t = cpool.tile([128, 3], dt.float32)
            nc.sync.dma_start(out=b1_t[:], in_=b1T[:])
            w1_t = cpool.tile([128, 4 * C], dt.bfloat16)
            nc.sync.dma_start(out=w1_t[0:C, :], in_=w1sb[:])
            w2_t = cpool.tile([128, 3, C], dt.bfloat16)
            nc.sync.dma_start(out=w2_t[:], in_=w2sb[:])

            # ---- SBUF fp32 accumulator ----
            acc_t = cpool.tile([128, NOT, C], dt.float32)
            nc.vector.memset(acc_t[:].rearrange("p a c -> p (a c)"), 0)

            # ---- phase 1 ----
            cur_gps = [None]

            for (wnd, t0, nt) in calls:
                n = nt * 128
                gi_t = st.tile([128, nt * 8], dt.int16, tag="gi")
                nc.sync.dma_start(out=gi_t[:],
                                  in_=gidxw[:, t0 * 8:(t0 + nt) * 8])
                ol_t = st.tile([128, nt], dt.bfloat16, tag="ol")
                nc.sync.dma_start(out=ol_t[:], in_=olocd[:, t0:t0 + nt])
                g_t = gkb.tile([128, TPC, CP], dt.bfloat16, tag="g")
                nc.gpsimd.dma_gather(g_t[:, 0:nt, :], ftab[wnd][:], gi_t[:],
                                     n, n, CP, single_packet=False)
                cb_t = cbp.tile([128, TPC, C], dt.bfloat16, tag="cb")

                kv_t = kvp.tile([1, TPC * 128], dt.float16, tag="kv")
                nc.sync.dma_start(out=kv_t[0:1, 0:n],
                                  in_=kval[:, t0 * 128:t0 * 128 + n])

                # per span: replicate k via K=1 matmul, k one-hots, w-expand
                for s0 in range(0, nt, SPAN):
                    sn = min(SPAN, nt - s0)
                    kbps = kbpp.tile([128, SPAN * 128], dt.float32, tag="kbps")
                    nc.tensor.matmul(out=kbps[:, 0:sn * 128],
                                     lhsT=ones1[:],
                                     rhs=kv_t[0:1, s0 * 128:(s0 + sn) * 128],
                                     start=True, stop=True)
                    oh_t = ohp.tile([128, 3, SPAN * 128], dt.bfloat16, tag="oh")
                    for c in range(3):
                        lo = hi = None
                        for j in range(sn):
                            if c in tile_chunks[t0 + s0 + j]:
                                lo = j if lo is None else lo
                                hi = j
                        if lo is None:
                            continue
                        nc.vector.tensor_tensor(
                            out=oh_t[:, c, lo * 128:(hi + 1) * 128],
                            in0=kbps[:, lo * 128:(hi + 1) * 128],
                            in1=ioc[:, c:c + 1].to_broadcast(
                                [128, (hi + 1 - lo) * 128]),
                            op=AL.is_equal)
                    for b0 in range(s0, s0 + sn, WPB):
                        bn = min(WPB, s0 + sn - b0)
                        wps = wpsp.tile([128, WPB, C], dt.float32, tag="wps")
                        for j in range(bn):
                            tj = t0 + b0 + j
                            chunks = tile_chunks[tj]
                            for ci, cch in enumerate(chunks):
                                nc.tensor.matmul(
                                    out=wps[:, j, :],
                                    lhsT=oh_t[:, cch,
                                              (b0 - s0 + j) * 128:
                                              (b0 - s0 + j + 1) * 128],
                                    rhs=wt[:, cch, :],
                                    start=(ci == 0),
                                    stop=(ci == len(chunks) - 1))
                        nc.vector.tensor_tensor(
                            out=cb_t[:, b0:b0 + bn, :],
                            in0=g_t[:, b0:b0 + bn, 0:C],
                            in1=wps[:, 0:bn, :],
                            op=AL.mult)

                # sel + group matmuls
                for s0 in range(0, nt, SELB):
                    sn = min(SELB, nt - s0)
                    sel_t = selp.tile([128, SELB, 128], dt.bfloat16, tag="sel")
                    nc.vector.tensor_tensor(
                        out=sel_t[:, 0:sn, :],
                        in0=ol_t[:, s0:s0 + sn].rearrange(
                            "p (t o) -> p t o", o=1).to_broadcast([128, sn, 128]),
                        in1=io128[:].rearrange(
                            "p (o f) -> p o f", o=1).to_broadcast([128, sn, 128]),
                        op=AL.is_equal)
                    for j in range(sn):
                        tj = t0 + s0 + j
                        if gs[tj]:
                            cur_gps[0] = gpsp.tile([128, C], dt.float32,
                                                   tag="gps", name="gps")
                        gps = cur_gps[0]
                        # find cb tile holding tj: cb pool tiles are per-call
                        nc.tensor.matmul(out=gps[:],
                                         lhsT=sel_t[:, j, :],
                                         rhs=cb_t[:, s0 + j, :],
                                         start=bool(gs[tj]),
                                         stop=bool(ge[tj]))
                        if ge[tj]:
                            ot = int(got[tj])
                            nc.vector.tensor_tensor(
                                out=acc_t[:, ot, :], in0=acc_t[:, ot, :],
                                in1=gps[:], op=AL.add)

            # ---- phase 2 ----
            for ot in range(NOT):
                f_t = p2.tile([128, C], dt.float32, tag="fres")
                nc.sync.dma_start(out=f_t[:],
                                  in_=fown[ot * 128:(ot + 1) * 128, :])
                x32 = p2.tile([128, C], dt.float32, tag="x32")
                nc.vector.tensor_tensor(out=x32[:], in0=acc_t[:, ot, :],
                                        in1=bdw_t[:], op=AL.add)
                sumx = p2.tile([128, 1], dt.float32, tag="sumx")
                nc.vector.tensor_reduce(out=sumx[:], in_=x32[:],
                                        axis=mybir.AxisListType.X, op=AL.add)
                mu = p2.tile([128, 1], dt.float32, tag="mu")
                nc.vector.tensor_scalar_mul(mu[:], sumx[:], 1.0 / C)
                sq_t = p2.tile([128, C], dt.float32, tag="sq")
                ssq = p2.tile([128, 1], dt.float32, tag="ssq")
                nc.scalar.activation(out=sq_t[:], in_=x32[:],
                                     func=AF.Square, accum_out=ssq[:])
                var = p2.tile([128, 1], dt.float32, tag="var")
                nc.vector.tensor_scalar_mul(var[:], ssq[:], 1.0 / C)
                mu2 = p2.tile([128, 1], dt.float32, tag="mu2")
                nc.vector.tensor_tensor(out=mu2[:], in0=mu[:], in1=mu[:],
                                        op=AL.mult)
                nc.vector.tensor_tensor(out=var[:], in0=var[:], in1=mu2[:],
                                        op=AL.subtract)
                nc.vector.tensor_scalar_add(var[:], var[:], EPS)
                std = p2.tile([128, 1], dt.float32, tag="std")
                nc.scalar.activation(out=std[:], in_=var[:], func=AF.Sqrt)
                rstd = p2.tile([128, 1], dt.float32, tag="rstd")
                nc.vector.reciprocal(rstd[:], std[:])
                xc = p2.tile([128, C], dt.float32, tag="xc")
                nc.vector.tensor_scalar(out=xc[:], in0=x32[:], scalar1=mu[:],
                                        scalar2=rstd[:],
                                        op0=AL.subtract, op1=AL.mult)
                nc.vector.tensor_tensor(out=xc[:], in0=xc[:], in1=gam_t[:],
                                        op=AL.mult)
                xln = p2.tile([128, C], dt.bfloat16, tag="xln")
                nc.vector.tensor_tensor(out=xln[:], in0=xc[:], in1=bet_t[:],
                                        op=AL.add)

                pst = p2ps.tile([128, 128], dt.bfloat16, tag="pst")
                nc.tensor.transpose(out=pst[0:C, :], in_=xln[:, 0:C],
                                    identity=ident_t[:])
                xT = p2.tile([128, 128], dt.bfloat16, tag="xT")
                nc.scalar.activation(out=xT[0:C, :], in_=pst[0:C, :],
                                     func=AF.Copy)

                psh = p2ps.tile([128, 3, 128], dt.float32, tag="psh")
                for k in range(3):
                    nc.tensor.matmul(
                        out=psh[:, k, :],
                        lhsT=w1_t[0:C, k * 128:(k + 1) * 128],
                        rhs=xT[0:C, :], start=True, stop=True)
                hT = p2.tile([128, 3, 128], dt.bfloat16, tag="hT")
                for k in range(3):
                    nc.scalar.activation(
                        out=hT[:, k, :], in_=psh[:, k, :], func=AF.Gelu,
                        bias=b1_t[:, k:k + 1], scale=1.0)

                psx = p2ps.tile([128, 128], dt.float32, tag="psx")
                for k in range(3):
                    nc.tensor.matmul(out=psx[:, 0:C], lhsT=hT[:, k, :],
                                     rhs=w2_t[:, k, :],
                                     start=(k == 0), stop=(k == 2))
                o32 = p2.tile([128, C], dt.float32, tag="o32")
                nc.vector.tensor_tensor(out=o32[:], in0=psx[:, 0:C],
                                        in1=b2_t[:], op=AL.add)
                nc.vector.tensor_tensor(out=o32[:], in0=o32[:], in1=f_t[:],
                                        op=AL.add)
                nc.sync.dma_start(out=outp[ot * 128:(ot + 1) * 128, :],
                                  in_=o32[:])
    nc.compile()
    return nc


# ---------------- public entry ----------------

_NC_CACHE = {}


def kernel(**inputs):
    global LAST_RESULT
    import os
    ncores_run = int(os.environ.get("KERNEL_NCORES", str(NCORE)))
    in_maps, sched = _prep(inputs)
    key = (sched["T_total"], tuple(sched["calls"]),
           tuple(sched["tile_chunks"]),
           sched["gs"].tobytes(), sched["ge"].tobytes(),
           sched["got"].tobytes())
    key = hash(key)
    if key not in _NC_CACHE:
        _NC_CACHE[key] = _build(sched)
    nc = _NC_CACHE[key]
    kw = {}
    if TRACE:
        kw.update(trace=True)
    res = run_bass_kernel_spmd(nc, in_maps[:ncores_run],
                               core_ids=list(range(ncores_run)), **kw)
    LAST_RESULT = res
    out = np.zeros((NV, C), np.float32)
    for c in range(ncores_run):
        out[c * VPC:(c + 1) * VPC] = res.results[c]["outp"][:VPC]
    return out


# revision 4
# speedup vs baseline: 1.7582x; 1.0029x over previous
"""Trainium2 Bass kernel v2 for nn_Block_19473381720396 (gnn_message_passing).

Key change vs baseline: the baseline spent 9.6ms/core of serialized GPSIMD
descriptor generation (3 SWDGE streams per edge: feats gather + w gather +
scatter-add, ~7.8ns/descriptor).  This version uses ONE SWDGE stream per edge
(the feats gather) and does everything else on PE/DVE/HWDGE:

  host: per core, edges sorted by (in-window, o-block, k); per (w, ot) group
        padded to a dataset-unified tile count; streams: gather idx (int16,
        wrap16 per call), o_local (bf16, slot-major), k (fp16 row).
  device phase 1, per gather call (<=TPC tiles, one in-window):
        HWDGE stream loads; partition_broadcast k -> kb [128, n]
        SWDGE dma_gather feats rows (one 256B desc per slot) -> g [slot,t,c]
        DVE tensor_scalar is_equal(kb, iota_c) -> k one-hot [k', slot]
        PE per tile: 1-3 matmuls oh_c^T @ wtab_c -> w_rows [slot, c] (PSUM)
        DVE mult g*w_rows -> contrib bf16
        DVE is_equal(oloc, iota) -> sel [slot, o_local]
        PE per tile: sel^T @ contrib accumulated into group PSUM [o_local, c]
        DVE per group: acc[:, ot, :] += gps   (SBUF fp32 acc, no HBM acc)
  device phase 2, per o-block: acc + b_dw, LayerNorm, transpose, MLP
        (w1/gelu/w2), + residual feats -> out.

No dma_scatter_add, no acc zero/reload DMA, no per-edge w gather.
"""
import sys

for _p in ("/opt/trn_rl_repo",):
    if _p not in sys.path:
        sys.path.insert(0, _p)

import numpy as np
import ml_dtypes

import concourse.bacc as bacc
import concourse.bass as bass
import concourse.mybir as mybir
import concourse.tile as tile
from concourse.bass_utils import run_bass_kernel_spmd

# ---------------- problem constants ----------------
NV = 200000        # voxels
C = 96             # channels
CP = 128           # padded channels (gather elem must be 256B multiple)
KV = 343           # kernel offsets
NCORE = 8
VPC = NV // NCORE  # 25000 out rows per core
W = 7              # in-windows (int16 gather index limit 32767)
WSZ = 28572        # window size (7*28572 >= 200000)
NOT = 196          # o-blocks of 128 rows (196*128 = 25088)
TPC = 32           # tiles per gather call (4096 slots)
SELB = 16          # sel build batch (tiles)
WPB = 4            # w-expand psum batch (tiles)
SPAN = 4           # oh span (tiles) -> [128, 3, 512]
EPS = 1e-6

TRACE = False
LAST_RESULT = None

_BF16 = ml_dtypes.bfloat16
_FP16 = np.float16


# ---------------- host-side prep ----------------

def _wrap16(a):
    """[n] -> [128, n//16]: slot j -> (j%16, j//16), replicated x8."""
    w = a.reshape(-1, 16).T
    return np.tile(w, (8, 1)).astype(np.int16)


def _prep(inputs):
    feats = np.asarray(inputs["feats"], np.float32)
    w_dw = np.asarray(inputs["w_dw"], np.float32)
    b_dw = np.asarray(inputs["b_dw"], np.float32)
    ln_w = np.asarray(inputs["ln_w"], np.float32)
    ln_b = np.asarray(inputs["ln_b"], np.float32)
    w1 = np.asarray(inputs["w1"], np.float32)
    b1 = np.asarray(inputs["b1"], np.float32)
    w2 = np.asarray(inputs["w2"], np.float32)
    b2 = np.asarray(inputs["b2"], np.float32)
    in_idx = np.asarray(inputs["in_idx"], np.int64)
    out_idx = np.asarray(inputs["out_idx"], np.int64)
    kernel_idx = np.asarray(inputs["kernel_idx"], np.int64)

    # shared tables
    ftabs = []
    for w in range(W):
        lo, hi = w * WSZ, min((w + 1) * WSZ, NV)
        t = np.zeros((WSZ, CP), _BF16)
        t[:hi - lo, :C] = feats[lo:hi].astype(_BF16)
        ftabs.append(t)
    wtab3 = np.zeros((128, 3, C), _BF16)
    wtab3f = np.zeros((384, C), np.float32)
    wtab3f[:KV] = w_dw
    for c in range(3):
        wtab3[:, c, :] = wtab3f[c * 128:(c + 1) * 128].astype(_BF16)

    iota128 = np.broadcast_to(np.arange(128, dtype=np.float32),
                              (128, 128)).astype(_BF16)
    iota128 = np.ascontiguousarray(iota128)
    iotac = np.zeros((128, 3), np.float32)
    for c in range(3):
        iotac[:, c] = np.arange(128) + 128 * c
    ident = np.eye(128, dtype=np.float32).astype(_BF16)
    bdwf = np.ascontiguousarray(np.broadcast_to(b_dw, (128, C)), np.float32)
    gamf = np.ascontiguousarray(np.broadcast_to(ln_w, (128, C)), np.float32)
    betf = np.ascontiguousarray(np.broadcast_to(ln_b, (128, C)), np.float32)
    b2f = np.ascontiguousarray(np.broadcast_to(b2, (128, C)), np.float32)
    b1T = np.ascontiguousarray(b1.reshape(3, 128).T, np.float32)
    w1sb = w1.astype(_BF16)
    w2sb = np.ascontiguousarray(
        w2.reshape(3, 128, C).transpose(1, 0, 2)).astype(_BF16)

    # ---- per-core edge partition + sort ----
    owner = out_idx // VPC
    cores = []
    for c in range(NCORE):
        sel = np.nonzero(owner == c)[0]
        eo = (out_idx[sel] - c * VPC).astype(np.int64)
        ei = in_idx[sel]
        ek = kernel_idx[sel]
        w = ei // WSZ
        wloc = (ei - w * WSZ).astype(np.int64)
        ot = eo >> 7
        ol = eo & 127
        order = np.lexsort((ek, ot, w))
        gid = (w[order] * NOT + ot[order]).astype(np.int64)
        cores.append({
            "gid": gid, "wloc": wloc[order], "ol": ol[order],
            "k": ek[order],
        })

    NG = W * NOT
    counts = np.zeros((NCORE, NG), np.int64)
    for c in range(NCORE):
        counts[c] = np.bincount(cores[c]["gid"], minlength=NG)
    tiles_c = (counts + 127) // 128
    tiles_uni = tiles_c.max(axis=0)          # unified tiles per group
    gtile0 = np.concatenate(([0], np.cumsum(tiles_uni)))  # group tile base
    T_total = int(gtile0[-1])
    S_total = T_total * 128
    # per-window tile counts + call split
    tw = tiles_uni.reshape(W, NOT).sum(1)
    calls = []  # (window, tile0, ntiles)
    tcur = 0
    for w in range(W):
        left = int(tw[w])
        while left > 0:
            n = min(TPC, left)
            calls.append((w, tcur, n))
            tcur += n
            left -= n
    assert tcur == T_total

    # ---- per-core slot arrays ----
    gbase_slots = gtile0[:-1] * 128
    kc_min = np.full((NCORE, T_total), 3, np.int64)
    kc_max = np.full((NCORE, T_total), -1, np.int64)
    per_core = []
    for c in range(NCORE):
        d = cores[c]
        gid = d["gid"]
        cnt = counts[c]
        starts = np.concatenate(([0], np.cumsum(cnt)))[:-1]
        rank = np.arange(len(gid)) - starts[gid]
        slot = gbase_slots[gid] + rank
        gidx = np.zeros(S_total, np.int16)
        gidx[slot] = d["wloc"]
        kvs = np.full(S_total, -1, np.int64)   # -1 = pad, fill later
        kvs[slot] = d["k"]
        olv = np.full(S_total, 255, np.int64)
        olv[slot] = d["ol"]
        # per-tile real k-chunk min/max
        kc = np.where(kvs >= 0, kvs >> 7, -1).reshape(T_total, 128)
        has = kc.max(1)
        kmax = kc.max(1)
        kcpos = np.where(kc < 0, 3, kc)
        kmin = kcpos.min(1)
        valid = has >= 0
        kc_min[c, valid] = kmin[valid]
        kc_max[c, valid] = kmax[valid]
        per_core.append({"gidx": gidx, "kvs": kvs, "olv": olv})

    cmin = kc_min.min(0)
    cmax = kc_max.max(0)
    empty = cmax < 0
    cmin[empty] = 2
    cmax[empty] = 2
    # pad k per tile = 128*cmin (valid wtab row; killed by sel anyway)
    padk = (cmin * 128).astype(np.int64)
    padk[empty] = KV  # zeros row
    tile_chunks = [tuple(range(int(cmin[t]), int(cmax[t]) + 1))
                   for t in range(T_total)]

    # group schedule: per tile -> (group-start?, group-end?, ot)
    gs = np.zeros(T_total, np.bool_)
    ge = np.zeros(T_total, np.bool_)
    got = np.zeros(T_total, np.int64)
    for g in range(NG):
        t0, t1 = int(gtile0[g]), int(gtile0[g + 1])
        if t1 > t0:
            gs[t0] = True
            ge[t1 - 1] = True
            got[t0:t1] = g % NOT
    sched = {
        "calls": calls, "tile_chunks": tile_chunks,
        "gs": gs, "ge": ge, "got": got, "T_total": T_total,
    }

    in_maps = []
    for c in range(NCORE):
        d = per_core[c]
        kvs = d["kvs"].reshape(T_total, 128)
        padm = kvs < 0
        kvs = np.where(padm, padk[:, None], kvs)
        kval = kvs.reshape(-1).astype(_FP16)[None, :]        # [1, S]
        oloc = np.ascontiguousarray(
            d["olv"].astype(np.float32).reshape(T_total, 128).T).astype(_BF16)
        # wrap idx per call
        gw = []
        for (w, t0, nt) in calls:
            seg = d["gidx"][t0 * 128:(t0 + nt) * 128]
            gw.append(_wrap16(seg))
        gidxw = np.ascontiguousarray(np.concatenate(gw, axis=1))  # [128, S/16]

        fown = np.zeros((NOT * 128, C), np.float32)
        fown[:VPC] = feats[c * VPC:(c + 1) * VPC]

        m = {
            "wtab3": wtab3, "iota128": iota128, "iotac": iotac, "ident": ident,
            "bdwf": bdwf, "gamf": gamf, "betf": betf, "b2f": b2f, "b1T": b1T,
            "w1sb": w1sb, "w2sb": w2sb, "fown": fown,
            "gidxw": gidxw, "kval": kval, "oloc": oloc,
        }
        for w in range(W):
            m[f"ftab{w}"] = ftabs[w]
        in_maps.append(m)
    return in_maps, sched


# ---------------- device program ----------------

def _build(sched):
    nc = bacc.Bacc("TRN2", target_bir_lowering=False, debug=False,
                   dynamic_dma_scratch_size=65536)
    dt = mybir.dt
    AL = mybir.AluOpType
    AF = mybir.ActivationFunctionType
    T_total = sched["T_total"]
    S_total = T_total * 128
    calls = sched["calls"]
    tile_chunks = sched["tile_chunks"]
    gs, ge, got = sched["gs"], sched["ge"], sched["got"]

    ftab = [nc.dram_tensor(f"ftab{w}", [WSZ, CP], dt.bfloat16,
                           kind="ExternalInput") for w in range(W)]
    wtab3 = nc.dram_tensor("wtab3", [128, 3, C], dt.bfloat16, kind="ExternalInput")
    iota128 = nc.dram_tensor("iota128", [128, 128], dt.bfloat16, kind="ExternalInput")
    iotac = nc.dram_tensor("iotac", [128, 3], dt.float32, kind="ExternalInput")
    ident = nc.dram_tensor("ident", [128, 128], dt.bfloat16, kind="ExternalInput")
    bdwf = nc.dram_tensor("bdwf", [128, C], dt.float32, kind="ExternalInput")
    gamf = nc.dram_tensor("gamf", [128, C], dt.float32, kind="ExternalInput")
    betf = nc.dram_tensor("betf", [128, C], dt.float32, kind="ExternalInput")
    b2f = nc.dram_tensor("b2f", [128, C], dt.float32, kind="ExternalInput")
    b1T = nc.dram_tensor("b1T", [128, 3], dt.float32, kind="ExternalInput")
    w1sb = nc.dram_tensor("w1sb", [C, 4 * C], dt.bfloat16, kind="ExternalInput")
    w2sb = nc.dram_tensor("w2sb", [128, 3, C], dt.bfloat16, kind="ExternalInput")
    fown = nc.dram_tensor("fown", [NOT * 128, C], dt.float32, kind="ExternalInput")
    gidxw = nc.dram_tensor("gidxw", [128, S_total // 16], dt.int16,
                           kind="ExternalInput")
    kval = nc.dram_tensor("kval", [1, S_total], dt.float16, kind="ExternalInput")
    olocd = nc.dram_tensor("oloc", [128, T_total], dt.bfloat16,
                           kind="ExternalInput")
    outp = nc.dram_tensor("outp", [NOT * 128, C], dt.float32,
                          kind="ExternalOutput")

    with tile.TileContext(nc) as tc:
        with tc.tile_pool(name="const", bufs=1) as cpool, \
             tc.tile_pool(name="st", bufs=3) as st, \
             tc.tile_pool(name="gkb", bufs=3) as gkb, \
             tc.tile_pool(name="kvp", bufs=2) as kvp, \
             tc.tile_pool(name="ohp", bufs=2) as ohp, \
             tc.tile_pool(name="cbp", bufs=2) as cbp, \
             tc.tile_pool(name="selp", bufs=2) as selp, \
             tc.tile_pool(name="p2", bufs=2) as p2, \
             tc.tile_pool(name="wpsp", bufs=2, space="PSUM") as wpsp, \
             tc.tile_pool(name="gpsp", bufs=2, space="PSUM") as gpsp, \
             tc.tile_pool(name="kbpp", bufs=1, space="PSUM") as kbpp, \
             tc.tile_pool(name="p2ps", bufs=1, space="PSUM") as p2ps:

            # ---- constants ----
            wt = cpool.tile([128, 3, C], dt.bfloat16)
            nc.sync.dma_start(out=wt[:], in_=wtab3[:])
            io128 = cpool.tile([128, 128], dt.bfloat16)
            nc.sync.dma_start(out=io128[:], in_=iota128[:])
            ioc = cpool.tile([128, 3], dt.float32)
            nc.sync.dma_start(out=ioc[:], in_=iotac[:])
            ones1 = cpool.tile([1, 128], dt.float16)
            nc.vector.memset(ones1[:], 1.0)
            ident_t = cpool.tile([128, 128], dt.bfloat16)
            nc.sync.dma_start(out=ident_t[:], in_=ident[:])
            bdw_t = cpool.tile([128, C], dt.float32)
            nc.sync.dma_start(out=bdw_t[:], in_=bdwf[:])
            gam_t = cpool.tile([128, C], dt.float32)
            nc.sync.dma_start(out=gam_t[:], in_=gamf[:])
            bet_t = cpool.tile([128, C], dt.float32)
            nc.sync.dma_start(out=bet_t[:], in_=betf[:])
            b2_t = cpool.tile([128, C], dt.float32)
            nc.sync.dma_start(out=b2_t[:], in_=b2f[:])
            b1_t = cpool.tile([128, 3], dt.float32)
            nc.sync.dma_start(out=b1_t[:], in_=b1T[:])
            w1_t = cpool.tile([128, 4 * C], dt.bfloat16)
            nc.sync.dma_start(out=w1_t[0:C, :], in_=w1sb[:])
            w2_t = cpool.tile([128, 3, C], dt.bfloat16)
            nc.sync.dma_start(out=w2_t[:], in_=w2sb[:])

            # ---- SBUF fp32 accumulator ----
            acc_t = cpool.tile([128, NOT, C], dt.float32)
            nc.vector.memset(acc_t[:].rearrange("p a c -> p (a c)"), 0)

            # ---- phase 1 ----
            cur_gps = [None]

            for (wnd, t0, nt) in calls:
                n = nt * 128
                gi_t = st.tile([128, nt * 8], dt.int16, tag="gi")
                nc.sync.dma_start(out=gi_t[:],
                                  in_=gidxw[:, t0 * 8:(t0 + nt) * 8])
                ol_t = st.tile([128, nt], dt.bfloat16, tag="ol")
                nc.sync.dma_start(out=ol_t[:], in_=olocd[:, t0:t0 + nt])
                g_t = gkb.tile([128, TPC, CP], dt.bfloat16, tag="g")
                nc.gpsimd.dma_gather(g_t[:, 0:nt, :], ftab[wnd][:], gi_t[:],
                                     n, n, CP, single_packet=False)
                cb_t = cbp.tile([128, TPC, C], dt.bfloat16, tag="cb")

                kv_t = kvp.tile([1, TPC * 128], dt.float16, tag="kv")
                nc.sync.dma_start(out=kv_t[0:1, 0:n],
                                  in_=kval[:, t0 * 128:t0 * 128 + n])

                # per span: replicate k via K=1 matmul, k one-hots, w-expand
                for s0 in range(0, nt, SPAN):
                    sn = min(SPAN, nt - s0)
                    kbps = kbpp.tile([128, SPAN * 128], dt.float32, tag="kbps")
                    nc.tensor.matmul(out=kbps[:, 0:sn * 128],
                                     lhsT=ones1[:],
                                     rhs=kv_t[0:1, s0 * 128:(s0 + sn) * 128],
                                     start=True, stop=True)
                    oh_t = ohp.tile([128, 3, SPAN * 128], dt.bfloat16, tag="oh")
                    for c in range(3):
                        lo = hi = None
                        for j in range(sn):
                            if c in tile_chunks[t0 + s0 + j]:
                                lo = j if lo is None else lo
                                hi = j
                        if lo is None:
                            continue
                        nc.vector.tensor_tensor(
                            out=oh_t[:, c, lo * 128:(hi + 1) * 128],
                            in0=kbps[:, lo * 128:(hi + 1) * 128],
                            in1=ioc[:, c:c + 1].to_broadcast(
                                [128, (hi + 1 - lo) * 128]),
                            op=AL.is_equal)
                    for b0 in range(s0, s0 + sn, WPB):
                        bn = min(WPB, s0 + sn - b0)
                        wps = wpsp.tile([128, WPB, C], dt.float32, tag="wps")
                        for j in range(bn):
                            tj = t0 + b0 + j
                            chunks = tile_chunks[tj]
                            for ci, cch in enumerate(chunks):
                                nc.tensor.matmul(
                                    out=wps[:, j, :],
                                    lhsT=oh_t[:, cch,
                                              (b0 - s0 + j) * 128:
                                              (b0 - s0 + j + 1) * 128],
                                    rhs=wt[:, cch, :],
                                    start=(ci == 0),
                                    stop=(ci == len(chunks) - 1))
                        nc.vector.tensor_tensor(
                            out=cb_t[:, b0:b0 + bn, :],
                            in0=g_t[:, b0:b0 + bn, 0:C],
                            in1=wps[:, 0:bn, :],
                            op=AL.mult)

                # sel + group matmuls
                for s0 in range(0, nt, SELB):
                    sn = min(SELB, nt - s0)
                    sel_t = selp.tile([128, SELB, 128], dt.bfloat16, tag="sel")
                    nc.vector.tensor_tensor(
                        out=sel_t[:, 0:sn, :],
                        in0=ol_t[:, s0:s0 + sn].rearrange(
                            "p (t o) -> p t o", o=1).to_broadcast([128, sn, 128]),
                        in1=io128[:].rearrange(
                            "p (o f) -> p o f", o=1).to_broadcast([128, sn, 128]),
                        op=AL.is_equal)
                    for j in range(sn):
                        tj = t0 + s0 + j
                        if gs[tj]:
                            cur_gps[0] = gpsp.tile([128, C], dt.float32,
                                                   tag="gps", name="gps")
                        gps = cur_gps[0]
                        # find cb tile holding tj: cb pool tiles are per-call
                        nc.tensor.matmul(out=gps[:],
                                         lhsT=sel_t[:, j, :],
                                         rhs=cb_t[:, s0 + j, :],
                                         start=bool(gs[tj]),
                                         stop=bool(ge[tj]))
                        if ge[tj]:
                            ot = int(got[tj])
                            nc.vector.tensor_tensor(
                                out=acc_t[:, ot, :], in0=acc_t[:, ot, :],
                                in1=gps[:], op=AL.add)

            # ---- phase 2 ----
            for ot in range(NOT):
                f_t = p2.tile([128, C], dt.float32, tag="fres")
                nc.sync.dma_start(out=f_t[:],
                                  in_=fown[ot * 128:(ot + 1) * 128, :])
                x32 = p2.tile([128, C], dt.float32, tag="x32")
                nc.vector.tensor_tensor(out=x32[:], in0=acc_t[:, ot, :],
                                        in1=bdw_t[:], op=AL.add)
                sumx = p2.tile([128, 1], dt.float32, tag="sumx")
                nc.vector.tensor_reduce(out=sumx[:], in_=x32[:],
                                        axis=mybir.AxisListType.X, op=AL.add)
                mu = p2.tile([128, 1], dt.float32, tag="mu")
                nc.vector.tensor_scalar_mul(mu[:], sumx[:], 1.0 / C)
                sq_t = p2.tile([128, C], dt.float32, tag="sq")
                ssq = p2.tile([128, 1], dt.float32, tag="ssq")
                nc.scalar.activation(out=sq_t[:], in_=x32[:],
                                     func=AF.Square, accum_out=ssq[:])
                var = p2.tile([128, 1], dt.float32, tag="var")
                nc.vector.tensor_scalar_mul(var[:], ssq[:], 1.0 / C)
                mu2 = p2.tile([128, 1], dt.float32, tag="mu2")
                nc.vector.tensor_tensor(out=mu2[:], in0=mu[:], in1=mu[:],
                                        op=AL.mult)
                nc.vector.tensor_tensor(out=var[:], in0=var[:], in1=mu2[:],
                                        op=AL.subtract)
                nc.vector.tensor_scalar_add(var[:], var[:], EPS)
                std = p2.tile([128, 1], dt.float32, tag="std")
                nc.scalar.activation(out=std[:], in_=var[:], func=AF.Sqrt)
                rstd = p2.tile([128, 1], dt.float32, tag="rstd")
                nc.vector.reciprocal(rstd[:], std[:])
                xc = p2.tile([128, C], dt.float32, tag="xc")
                nc.vector.tensor_scalar(out=xc[:], in0=x32[:], scalar1=mu[:],
                                        scalar2=rstd[:],
                                        op0=AL.subtract, op1=AL.mult)
                nc.vector.tensor_tensor(out=xc[:], in0=xc[:], in1=gam_t[:],
                                        op=AL.mult)
                xln = p2.tile([128, C], dt.bfloat16, tag="xln")
                nc.vector.tensor_tensor(out=xln[:], in0=xc[:], in1=bet_t[:],
                                        op=AL.add)

                pst = p2ps.tile([128, 128], dt.bfloat16, tag="pst")
                nc.tensor.transpose(out=pst[0:C, :], in_=xln[:, 0:C],
                                    identity=ident_t[:])
                xT = p2.tile([128, 128], dt.bfloat16, tag="xT")
                nc.scalar.activation(out=xT[0:C, :], in_=pst[0:C, :],
                                     func=AF.Copy)

                psh = p2ps.tile([128, 3, 128], dt.float32, tag="psh")
                for k in range(3):
                    nc.tensor.matmul(
                        out=psh[:, k, :],
                        lhsT=w1_t[0:C, k * 128:(k + 1) * 128],
                        rhs=xT[0:C, :], start=True, stop=True)
                hT = p2.tile([128, 3, 128], dt.bfloat16, tag="hT")
                for k in range(3):
                    nc.scalar.activation(
                        out=hT[:, k, :], in_=psh[:, k, :], func=AF.Gelu,
                        bias=b1_t[:, k:k + 1], scale=1.0)

                psx = p2ps.tile([128, 128], dt.float32, tag="psx")
                for k in range(3):
                    nc.tensor.matmul(out=psx[:, 0:C], lhsT=hT[:, k, :],
                                     rhs=w2_t[:, k, :],
                                     start=(k == 0), stop=(k == 2))
                o32 = p2.tile([128, C], dt.float32, tag="o32")
                nc.vector.tensor_tensor(out=o32[:], in0=psx[:, 0:C],
                                        in1=b2_t[:], op=AL.add)
                nc.vector.tensor_tensor(out=o32[:], in0=o32[:], in1=f_t[:],
                                        op=AL.add)
                nc.sync.dma_start(out=outp[ot * 128:(ot + 1) * 128, :],
                                  in_=o32[:])
    nc.compile()
    return nc


# ---------------- public entry ----------------

_NC_CACHE = {}


def kernel(**inputs):
    global LAST_RESULT
    import os
    ncores_run = int(os.environ.get("KERNEL_NCORES", str(NCORE)))
    in_maps, sched = _prep(inputs)
    key = (sched["T_total"], tuple(sched["calls"]),
           tuple(sched["tile_chunks"]),
           sched["gs"].tobytes(), sched["ge"].tobytes(),
           sched["got"].tobytes())
    key = hash(key)
    if key not in _NC_CACHE:
        _NC_CACHE[key] = _build(sched)
    nc = _NC_CACHE[key]
    kw = {}
    if TRACE:
        kw.update(trace=True)
    res = run_bass_kernel_spmd(nc, in_maps[:ncores_run],
                               core_ids=list(range(ncores_run)), **kw)
    LAST_RESULT = res
    out = np.zeros((NV, C), np.float32)
    for c in range(ncores_run):
        out[c * VPC:(c + 1) * VPC] = res.results[c]["outp"][:VPC]
    return out
